# revision 48
# baseline (speedup 1.0000x reference)
"""Trainium2 Bass kernel for AttentionWithComplexRoPE.

Strategy (8 NeuronCores): data-parallel over batch (B=2) x tensor-parallel
over heads (16 heads -> 4 per core). Core c handles batch c//4, heads
[4*(c%4), 4*(c%4)+4).

Per-core pipeline (K-packed attention, ~226 us modeled vs 432 us baseline):
  phase 1: q/k projections (bf16 x/W, f32 PSUM) in deinterleaved layout
           [128 = 4h x 32 re|im feats, t]; RoPE as wide DVE elementwise
           ops; results repacked (Act+Pool 32-row copies) into PAIR-PACKED
           tiles qri/kri[X] [128 = 2 heads x (re32|im32), t] so QK runs as
           ONE K=64 fp32r matmul per head (1 cycle/row — half the PE time
           of the re/im-split form). v in [t, f] layout with a ones column
           (softmax-denominator trick). DMA issue order puts k-weights and
           chunk-0 x first so the PE starts at ~7 us.
  phase 2: flat software pipeline over (query-chunk s0, key-tile tt): per
           step 2 head-groups, each 2 K=64 QK matmuls -> [128, 1024] PSUM
           (2 single-buffered tag tiles) -> Exp on ScalarE (1/8 scale
           folded) -> 4 PV matmuls (M=65, K=128) accumulating over tt into
           4 [65, 512] PSUM accs. PV of step i-1 is emitted after QK of
           step i so the PE never head-of-line blocks the Act engine; the
           Act engine runs exp back-to-back (~1.04 us per 1024-wide tile)
           and is the phase bottleneck (its 0.83 ns/col throughput is the
           kernel's hard floor: B*H*S^2/8 cols/core). Softmax normalize on
           DVE (recip, mult) + gpsimd (partition broadcast) into
           head-pair-packed att2[X] [128 = 2 heads x 64 d, t] (bf16).
  phase 3: Wo projection with K=128 (head-pair packed, bf16 weights),
           PSUM->SBUF eviction split DVE/Act, f16 output rows DMA'd on
           alternating queues (sync/scalar).
Host: slice/permute weights (bf16), transpose x (bf16), replicate freqs
(f32); sum the 4 per-batch f16 partials in f32 at the end.

PSUM budget: ph1 proj(4) | qk tags(2x2) + accs(4) | ph3 y(4) — 8 banks.
"""
import sys

if "/opt/trn_rl_repo" not in sys.path:
    sys.path.insert(0, "/opt/trn_rl_repo")

import ml_dtypes
import numpy as np

import concourse.bass as bass
import concourse.mybir as mybir
import concourse.tile as tile
from concourse import bacc
from concourse.bass_utils import run_bass_kernel_spmd

F32 = mybir.dt.float32
F32R = mybir.dt.float32r
BF16 = mybir.dt.bfloat16
F16 = mybir.dt.float16

B, S, C = 2, 2048, 1024
H = 16                      # global heads
HL = 4                      # heads per core
D = C // H                  # 64
DH = 32                     # complex pairs per head
F = HL * D                  # 256 local features
N_CORES = 8
KT = C // 128               # 8 contraction tiles for projections
TT = S // 128               # 16 token tiles
SC = S // 512               # 4 s-chunks
SCALE = float(D) ** -0.5

_CACHED_NC = None


def build_module():
    nc = bacc.Bacc("TRN2", target_bir_lowering=False)

    xt = nc.dram_tensor("xt", [C, S], BF16, kind="ExternalInput")
    wqr = nc.dram_tensor("wqr", [128, KT * 128], BF16, kind="ExternalInput")
    wqi = nc.dram_tensor("wqi", [128, KT * 128], BF16, kind="ExternalInput")
    wkr = nc.dram_tensor("wkr", [128, KT * 128], BF16, kind="ExternalInput")
    wki = nc.dram_tensor("wki", [128, KT * 128], BF16, kind="ExternalInput")
    wv = nc.dram_tensor("wv", [128, KT * 256], BF16, kind="ExternalInput")
    wo = nc.dram_tensor("wo", [2, 128, C], BF16, kind="ExternalInput")
    fr = nc.dram_tensor("fr", [128, S], F32, kind="ExternalInput")
    fi = nc.dram_tensor("fi", [128, S], F32, kind="ExternalInput")
    out = nc.dram_tensor("out", [S, C], F16, kind="ExternalOutput")

    with tile.TileContext(nc) as tc:
        with tc.tile_pool(name="persist", bufs=1) as persist:
            # persistent sbuf tensors
            qri = [persist.tile([128, S], F32R, name=f"qri{x}")
                   for x in range(2)]
            kri = [persist.tile([128, S], F32R, name=f"kri{x}")
                   for x in range(2)]
            vaug_sb = persist.tile([128, TT, HL * 65], F32R)  # v + ones col
            att2 = [persist.tile([128, S], BF16, name=f"att2_{x}")
                    for x in range(2)]
            wo_sb = [persist.tile([128, C], BF16, name=f"wo{x}_sb")
                     for x in range(2)]

            # ones columns of v_aug (col 64 of each head block), all at once
            nc.vector.memset(
                vaug_sb.rearrange("p tt (h d) -> p tt h d", h=HL)
                [:, :, :, 64:65].bitcast(F32), 1.0)

            # ---------------- phase 1: projections + rope ----------------
            xt_r = xt.rearrange("(kt p) s -> p kt s", p=128)

            # Attention pools span phase 1 so chunk-0 attention (s0=0) can
            # interleave with the remaining projection chunks. PSUM budget
            # during the overlap: ph1ps(2) + qke(2) + accps(4) = 8 banks.
            with tc.tile_pool(name="ph2", bufs=4) as ph2, \
                 tc.tile_pool(name="ph2s", bufs=4) as ph2s, \
                 tc.tile_pool(name="accps", bufs=4, space="PSUM") as accps:
                st2 = {"accs": None, "pv": None}

                def emit_pv(pend):
                    ptt, ps0, paccs, pexps = pend
                    for h in range(HL):
                        nc.tensor.matmul(
                            paccs[h],
                            vaug_sb[:, ptt, 65 * h:65 * (h + 1)],
                            pexps[h // 2][:, 512 * (h % 2):512 * (h % 2 + 1)],
                            start=(ptt == 0), stop=(ptt == TT - 1))

                def emit_normalize(s0, paccs):
                    # att2[h//2][64*(h%2):+64, ssl] = acc[0:64] * (1/acc[64])
                    ssl = slice(512 * s0, 512 * (s0 + 1))
                    for h in range(HL):
                        recip = ph2s.tile([1, 512], F32, tag="recip")
                        nc.vector.reciprocal(recip, paccs[h][64:65, :])
                        bcast = ph2s.tile([64, 512], F32, tag="bcast")
                        nc.gpsimd.partition_broadcast(bcast, recip)
                        po = 64 * (h % 2)
                        nc.vector.tensor_tensor(
                            att2[h // 2][po:po + 64, ssl],
                            paccs[h][0:64, :], bcast,
                            op=mybir.AluOpType.mult)

                def emit_step(s0, tt, qk_alloc):
                    # QK + exp for (s0, tt); then PV (and normalize at chunk
                    # wrap) for the PREVIOUS step so the PE never sits
                    # head-of-line blocked on this step's exp.
                    ssl = slice(512 * s0, 512 * (s0 + 1))
                    tsl = slice(128 * tt, 128 * (tt + 1))
                    if tt == 0:
                        st2["accs"] = [accps.tile([65, 512], F32, tag="acc",
                                                  name=f"acc{s0}_{h}")
                                       for h in range(HL)]
                    exps_g = []
                    for g in range(2):
                        qk = qk_alloc(g)
                        for hh in range(2):
                            po = 64 * hh
                            nc.tensor.matmul(
                                qk[:, 512 * hh:512 * (hh + 1)],
                                kri[g][po:po + 64, tsl],
                                qri[g][po:po + 64, ssl],
                                start=True, stop=True,
                                tile_position=(po, 0))
                        exps = ph2.tile([128, 1024], F32R, tag=f"ex{g}",
                                        bufs=2)
                        nc.scalar.activation(
                            exps, qk, mybir.ActivationFunctionType.Exp,
                            scale=SCALE)
                        exps_g.append(exps)
                    if st2["pv"] is not None:
                        emit_pv(st2["pv"])
                        if st2["pv"][0] == TT - 1:
                            emit_normalize(st2["pv"][1], st2["pv"][2])
                    st2["pv"] = (tt, s0, st2["accs"], exps_g)

                with tc.tile_pool(name="ph1", bufs=2) as ph1, \
                     tc.tile_pool(name="ph1ps", bufs=4, space="PSUM") as ph1ps, \
                     tc.tile_pool(name="ropet", bufs=2) as ropet:
                    # DMA issue order = the phase-1 critical path: k weights,
                    # then chunk-0 x/freqs, then the remaining weights.
                    w_sb = {}
                    for nm, dram in (("kr", wkr), ("ki", wki)):
                        w = ph1.tile([128, KT * 128], BF16, name=f"w_{nm}",
                                     tag=nm, bufs=1)
                        nc.sync.dma_start(out=w, in_=dram.ap())
                        w_sb[nm] = w
                    xtqs, frs, fis = [], [], []
                    for s0 in range(SC):
                        sl = slice(512 * s0, 512 * (s0 + 1))
                        xtq = ph1.tile([128, KT, 512], BF16, tag="xtq",
                                       bufs=2)
                        nc.sync.dma_start(out=xtq, in_=xt_r[:, :, sl])
                        fr_sb = ph1.tile([128, 512], F32, tag="fr", bufs=2)
                        fi_sb = ph1.tile([128, 512], F32, tag="fi", bufs=2)
                        nc.sync.dma_start(out=fr_sb, in_=fr.ap()[:, sl])
                        nc.sync.dma_start(out=fi_sb, in_=fi.ap()[:, sl])
                        xtqs.append(xtq)
                        frs.append(fr_sb)
                        fis.append(fi_sb)
                        if s0 == 0:
                            for nm, dram in (("qr", wqr), ("qi", wqi)):
                                w = ph1.tile([128, KT * 128], BF16,
                                             name=f"w_{nm}", tag=nm, bufs=1)
                                nc.sync.dma_start(out=w, in_=dram.ap())
                                w_sb[nm] = w
                            wv_sb = ph1.tile([128, KT * 256], BF16, tag="wv",
                                             bufs=1)
                            nc.sync.dma_start(out=wv_sb, in_=wv.ap())
                            for x in range(2):
                                nc.sync.dma_start(out=wo_sb[x],
                                                  in_=wo.ap()[x])

                    def do_chunk(s0, steps=()):
                        steps = list(steps)
                        sl = slice(512 * s0, 512 * (s0 + 1))
                        xtq, fr_sb, fi_sb = xtqs[s0], frs[s0], fis[s0]
                        # k and q projections + rope for this token chunk
                        for nm, dsts in (("k", kri), ("q", qri)):
                            wr_, wi_ = w_sb[nm + "r"], w_sb[nm + "i"]
                            ps_r = ph1ps.tile([128, 512], F32, tag="proj")
                            ps_i = ph1ps.tile([128, 512], F32, tag="proj")
                            for kt in range(KT):
                                nc.tensor.matmul(
                                    ps_r, wr_[:, 128 * kt:128 * (kt + 1)],
                                    xtq[:, kt, :],
                                    start=(kt == 0), stop=(kt == KT - 1))
                            for kt in range(KT):
                                nc.tensor.matmul(
                                    ps_i, wi_[:, 128 * kt:128 * (kt + 1)],
                                    xtq[:, kt, :],
                                    start=(kt == 0), stop=(kt == KT - 1))
                            # rope: r' = r*fr - i*fi ; i' = r*fi + i*fr
                            t_rr = ropet.tile([128, 512], F32, tag="t0")
                            t_ii = ropet.tile([128, 512], F32, tag="t1")
                            t_ri = ropet.tile([128, 512], F32, tag="t2")
                            t_ir = ropet.tile([128, 512], F32, tag="t3")
                            nc.vector.tensor_tensor(t_rr, ps_r, fr_sb,
                                                    op=mybir.AluOpType.mult)
                            nc.vector.tensor_tensor(t_ii, ps_i, fi_sb,
                                                    op=mybir.AluOpType.mult)
                            nc.vector.tensor_tensor(t_ri, ps_r, fi_sb,
                                                    op=mybir.AluOpType.mult)
                            nc.vector.tensor_tensor(t_ir, ps_i, fr_sb,
                                                    op=mybir.AluOpType.mult)
                            ro = ropet.tile([128, 512], F32, tag="ro")
                            io = ropet.tile([128, 512], F32, tag="io")
                            nc.vector.tensor_tensor(
                                ro, t_rr, t_ii, op=mybir.AluOpType.subtract)
                            nc.vector.tensor_tensor(
                                io, t_ri, t_ir, op=mybir.AluOpType.add)
                            # repack into pair tiles, split Act/Pool engines
                            # (32-row copies: the packed layout interleaves
                            # re/im halves; strided partition APs don't
                            # compile)
                            for x in range(2):
                                dst = dsts[x][:, sl]
                                for hh in range(2):
                                    nc.scalar.copy(
                                        dst[64 * hh:64 * hh + 32, :],
                                        ro[64 * x + 32 * hh:
                                           64 * x + 32 * (hh + 1), :])
                                    nc.gpsimd.tensor_copy(
                                        dst[64 * hh + 32:64 * (hh + 1), :],
                                        io[64 * x + 32 * hh:
                                           64 * x + 32 * (hh + 1), :])
                            for th in steps[:2]:
                                th()
                            del steps[:2]

                        # v projection into [t, f] with ones cols interleaved
                        for tl in range(4):
                            tt = 4 * s0 + tl
                            ps_v = ph1ps.tile([128, 256], F32, tag="proj")
                            for kt in range(KT):
                                nc.tensor.matmul(
                                    ps_v, xtq[:, kt, 128 * tl:128 * (tl + 1)],
                                    wv_sb[:, 256 * kt:256 * (kt + 1)],
                                    start=(kt == 0), stop=(kt == KT - 1))
                            # strided evict: head h -> cols [65h, 65h+64)
                            vv = vaug_sb[:, tt, :].rearrange(
                                "p (h d) -> p h d", h=HL)
                            nc.scalar.copy(
                                vv[:, :, 0:64],
                                ps_v.rearrange("p (h d) -> p h d", h=HL))
                        for th in steps:
                            th()

                    for c in range(SC):
                        do_chunk(c)

                # ------------- attention for chunks 1..3 ----------------
                with tc.tile_pool(name="qkps", bufs=1,
                                  space="PSUM") as qkps:
                    def qk_alloc(g):
                        return qkps.tile([128, 1024], F32, tag=f"qk{g}",
                                         bufs=1, name=f"qk_{g}")
                    for i in range(SC * TT):
                        s0, tt = divmod(i, TT)
                        emit_step(s0, tt, qk_alloc)
                    emit_pv(st2["pv"])
                    emit_normalize(st2["pv"][1], st2["pv"][2])

                # ---------------- phase 3: output projection --------------
                with tc.tile_pool(name="ph3", bufs=4) as ph3, \
                     tc.tile_pool(name="ph3ps", bufs=4, space="PSUM") as ph3ps:
                    for st in range(TT):
                        tsl = slice(128 * st, 128 * (st + 1))
                        y_sb = ph3.tile([128, C], F16, tag="y_sb", bufs=4)
                        for cc in range(C // 512):
                            csl = slice(512 * cc, 512 * (cc + 1))
                            ps_y = ph3ps.tile([128, 512], F32, tag="y")
                            for x in range(2):
                                nc.tensor.matmul(
                                    ps_y, att2[x][:, tsl],
                                    wo_sb[x][:, csl],
                                    start=(x == 0), stop=(x == 1))
                            if cc == 0:
                                nc.vector.tensor_copy(y_sb[:, csl], ps_y)
                            else:
                                nc.scalar.copy(y_sb[:, csl], ps_y)
                        eng = nc.sync if st % 2 == 0 else nc.scalar
                        eng.dma_start(out=out.ap()[tsl, :], in_=y_sb)

    nc.compile()
    return nc


def make_inputs(x, freqs, Wq, Wk, Wv, Wo):
    """Build the 8 per-core input maps."""
    rnd = lambda a: np.ascontiguousarray(a, dtype=ml_dtypes.bfloat16)  # noqa: E731

    # deinterleave permutations of the 256 local feature rows
    p = np.arange(128)
    real_rows = 64 * (p // 32) + 2 * (p % 32)       # within local 256 block
    imag_rows = real_rows + 1

    frh = np.ascontiguousarray(np.tile(freqs[:, :, 0].T, (HL, 1)),
                               dtype=np.float32)    # [128, S]
    fih = np.ascontiguousarray(np.tile(freqs[:, :, 1].T, (HL, 1)),
                               dtype=np.float32)

    def proj_weight(W, rows):
        # lhsT tiles: [128 c-part, KT*128], w[p, kt*128+m] = W[base+rows[m], kt*128+p]
        wt = W[rows, :]                              # [128, C]
        return rnd(wt.T.reshape(KT, 128, 128).transpose(1, 0, 2)
                   .reshape(128, KT * 128))

    in_maps = []
    for c in range(N_CORES):
        b, hg = divmod(c, 4)
        base = 256 * hg
        wqr = proj_weight(Wq, base + real_rows)
        wqi = proj_weight(Wq, base + imag_rows)
        wkr = proj_weight(Wk, base + real_rows)
        wki = proj_weight(Wk, base + imag_rows)
        # v: [128 c-part, KT*256], wv[p, kt*256+f] = Wv[base+f, kt*128+p]
        wvt = Wv[base:base + F, :].T                 # [C, F]
        wv_ = rnd(wvt.reshape(KT, 128, F).transpose(1, 0, 2)
                  .reshape(128, KT * F))
        # wo: [2, 128, C]; pair tile x rows = Wo columns for heads 2x,2x+1
        wo_ = np.empty((2, 128, C), np.float32)
        for xx in range(2):
            wo_[xx] = Wo[:, base + 128 * xx: base + 128 * (xx + 1)].T
        in_maps.append({
            "xt": rnd(x[b].T),
            "wqr": wqr, "wqi": wqi, "wkr": wkr, "wki": wki,
            "wv": wv_, "wo": rnd(wo_),
            "fr": frh, "fi": fih,
        })
    return in_maps


def kernel(x, freqs, Wq, Wk, Wv, Wo):
    global _CACHED_NC
    x = np.asarray(x, dtype=np.float32)
    freqs = np.asarray(freqs, dtype=np.float32)
    Wq = np.asarray(Wq, dtype=np.float32)
    Wk = np.asarray(Wk, dtype=np.float32)
    Wv = np.asarray(Wv, dtype=np.float32)
    Wo = np.asarray(Wo, dtype=np.float32)

    in_maps = make_inputs(x, freqs, Wq, Wk, Wv, Wo)
    if _CACHED_NC is None:
        _CACHED_NC = build_module()
    res = run_bass_kernel_spmd(_CACHED_NC, in_maps,
                               core_ids=list(range(N_CORES)))
    outs = [np.asarray(r["out"], dtype=np.float32) for r in res.results]
    y = np.empty((B, S, C), np.float32)
    for b in range(B):
        y[b] = outs[4 * b] + outs[4 * b + 1] + outs[4 * b + 2] + outs[4 * b + 3]
    return y


if __name__ == "__main__":
    rng = np.random.default_rng(0)
    x = rng.standard_normal((B, S, C)).astype(np.float32)
    freqs = rng.standard_normal((S, DH, 2)).astype(np.float32)
    ws = [(rng.standard_normal((C, C)) * C ** -0.5).astype(np.float32)
          for _ in range(4)]
    y = kernel(x, freqs, *ws)
    print("out", y.shape, y.dtype, float(np.abs(y).mean()))


# revision 53
# speedup vs baseline: 1.0082x; 1.0082x over previous
"""Trainium2 Bass kernel for AttentionWithComplexRoPE.

Strategy (8 NeuronCores): data-parallel over batch (B=2) x tensor-parallel
over heads (16 heads -> 4 per core). Core c handles batch c//4, heads
[4*(c%4), 4*(c%4)+4).

Per-core pipeline (~224 us modeled vs 432 us baseline):
  phase 1: q/k projections (bf16 x/W, f32 PSUM); RoPE as wide DVE
           elementwise ops; results repacked (Act+Pool 32-row copies) into
           PAIR-PACKED tiles qri/kri[X] [128 = 2 heads x (re32|im32), t]
           so QK runs as ONE K=64 fp32r matmul per head. v in [t, f]
           layout with a ones column (softmax-denominator trick). DMA
           issue order puts k-weights and chunk-0 x first.
  phase 2+3 (fused): flat software pipeline over (query-chunk s0, key-tile
           tt). Per step: 4 K=64 QK matmuls -> [128,1024] PSUM x2 tags ->
           Exp on ScalarE (bf16 out, 4-deep buffers) -> 16 PV matmuls with
           QUERIES ON THE OUTPUT PARTITION DIM (lhsT = exp tile slice,
           moving = v_aug [128,65]; 65-row cost at full K=128/M=128). PV
           accumulates into 4 [128, 4hx65] acc tiles; col 64 of each head
           block is the softmax denominator -> a per-PARTITION scalar.
           NOTE: the PSUM `start` flag zeroes the whole BANK, so only the
           first region-write of an acc/transpose tile carries start=True.
           At each chunk wrap: normalize via DVE reciprocal +
           tensor_scalar_mul, PE-transpose (via identity) back into the
           [d, t] pair layout att2, then the Wo projection + f16 output
           DMA ride the same accps rotation (no separate phase-3 tail).
           The Act engine's exp stream (0.83 ns/col, B*H*S^2/8 cols/core)
           is the kernel's hard floor.
Host: slice/permute weights (bf16), transpose x (bf16), replicate freqs
(f32), identity for PE transpose; sum the 4 per-batch f16 partials in f32.

PSUM budget: ph1 proj(4) | qk tags(2x2) + acc/transpose/Wo rotation(4).
"""
import sys

if "/opt/trn_rl_repo" not in sys.path:
    sys.path.insert(0, "/opt/trn_rl_repo")

import ml_dtypes
import numpy as np

import concourse.bass as bass
import concourse.mybir as mybir
import concourse.tile as tile
from concourse import bacc
from concourse.bass_utils import run_bass_kernel_spmd

F32 = mybir.dt.float32
F32R = mybir.dt.float32r
BF16 = mybir.dt.bfloat16
F16 = mybir.dt.float16

B, S, C = 2, 2048, 1024
H = 16                      # global heads
HL = 4                      # heads per core
D = C // H                  # 64
DH = 32                     # complex pairs per head
F = HL * D                  # 256 local features
N_CORES = 8
KT = C // 128               # 8 contraction tiles for projections
TT = S // 128               # 16 token tiles
SC = S // 512               # 4 s-chunks
SCALE = float(D) ** -0.5

_CACHED_NC = None


def build_module():
    nc = bacc.Bacc("TRN2", target_bir_lowering=False)

    xt = nc.dram_tensor("xt", [C, S], BF16, kind="ExternalInput")
    wqr = nc.dram_tensor("wqr", [128, KT * 128], BF16, kind="ExternalInput")
    wqi = nc.dram_tensor("wqi", [128, KT * 128], BF16, kind="ExternalInput")
    wkr = nc.dram_tensor("wkr", [128, KT * 128], BF16, kind="ExternalInput")
    wki = nc.dram_tensor("wki", [128, KT * 128], BF16, kind="ExternalInput")
    wv = nc.dram_tensor("wv", [128, KT * 256], BF16, kind="ExternalInput")
    wo = nc.dram_tensor("wo", [2, 128, C], BF16, kind="ExternalInput")
    fr = nc.dram_tensor("fr", [128, S], F32, kind="ExternalInput")
    fi = nc.dram_tensor("fi", [128, S], F32, kind="ExternalInput")
    ident = nc.dram_tensor("ident", [128, 128], BF16, kind="ExternalInput")
    out = nc.dram_tensor("out", [S, C], F16, kind="ExternalOutput")

    with tile.TileContext(nc) as tc:
        with tc.tile_pool(name="persist", bufs=1) as persist:
            # persistent sbuf tensors
            qri = [persist.tile([128, S], F32R, name=f"qri{x}")
                   for x in range(2)]
            kri = [persist.tile([128, S], F32R, name=f"kri{x}")
                   for x in range(2)]
            vaug_sb = persist.tile([128, TT, HL * 65], BF16)  # v + ones col
            ident_sb = persist.tile([128, 128], BF16)
            ident32_sb = persist.tile([128, 128], F32)
            att2 = [persist.tile([128, S], BF16, name=f"att2_{x}")
                    for x in range(2)]
            wo_sb = [persist.tile([128, C], BF16, name=f"wo{x}_sb")
                     for x in range(2)]

            # ones columns of v_aug (col 64 of each head block), all at once
            nc.vector.memset(
                vaug_sb.rearrange("p tt (h d) -> p tt h d", h=HL)
                [:, :, :, 64:65], 1.0)

            # ---------------- phase 1: projections + rope ----------------
            xt_r = xt.rearrange("(kt p) s -> p kt s", p=128)

            # Attention pools span phase 1 so chunk-0 attention (s0=0) can
            # interleave with the remaining projection chunks. PSUM budget
            # during the overlap: ph1ps(2) + qke(2) + accps(4) = 8 banks.
            with tc.tile_pool(name="ph2", bufs=4) as ph2, \
                 tc.tile_pool(name="ph2s", bufs=4) as ph2s, \
                 tc.tile_pool(name="accps", bufs=4, space="PSUM") as accps:
                st2 = {"accs": None, "pv": None, "ph3": None}

                def emit_pv(pend):
                    # PV with queries on the output PARTITION dim: lhsT is
                    # the exp tile slice [128 keys, 128 queries], moving is
                    # v_aug [128 keys, 65] -> 65-row matmuls at full K=128,
                    # M=128 PE utilization (bf16). acc[qt] = [128 q, 4h x 65]
                    # accumulated over key tiles; col 64 of each head block
                    # is the softmax denominator (a per-PARTITION scalar).
                    ptt, ps0, pexps = pend
                    if ptt == 0:
                        st2["accs"] = [accps.tile([128, HL * 65], F32,
                                                  tag="acc",
                                                  name=f"acc{ps0}_{qt}")
                                       for qt in range(4)]
                    paccs = st2["accs"]
                    for qt in range(4):
                        for h in range(HL):
                            po = 512 * (h % 2) + 128 * qt
                            nc.tensor.matmul(
                                paccs[qt][:, 65 * h:65 * (h + 1)],
                                pexps[h // 2][:, po:po + 128],
                                vaug_sb[:, ptt, 65 * h:65 * (h + 1)],
                                start=(ptt == 0 and h == 0),
                                stop=(ptt == TT - 1),
                                skip_group_check=True)

                def emit_fin_a(s0):
                    # softmax normalize (per-partition reciprocals) and PE
                    # transpose back into the [d, t] pair layout att2.
                    paccs = st2["accs"]
                    ssl = slice(512 * s0, 512 * (s0 + 1))
                    att_qts = []
                    for qt in range(4):
                        acc = paccs[qt]
                        accv = acc.rearrange("p (h e) -> p h e", h=HL)
                        recip4 = ph2s.tile([128, HL], F32, tag="recip")
                        nc.vector.reciprocal(recip4, accv[:, :, 64])
                        att_qt = ph2s.tile([128, F], F32, tag="attq",
                                           bufs=4)
                        for h in range(HL):
                            nc.vector.tensor_scalar_mul(
                                att_qt[:, 64 * h:64 * (h + 1)],
                                acc[:, 65 * h:65 * h + 64],
                                recip4[:, h:h + 1])
                        att_qts.append(att_qt)
                    for x in range(2):
                        tp = accps.tile([128, 512], F32, tag="acc",
                                        name=f"tp{x}")
                        for qt in range(4):
                            nc.tensor.matmul(
                                tp[:, 128 * qt:128 * (qt + 1)],
                                att_qts[qt][:, 128 * x:128 * (x + 1)],
                                ident32_sb, is_transpose=True,
                                start=(qt == 0), stop=(qt == 3),
                                skip_group_check=True)
                        nc.vector.tensor_copy(att2[x][:, ssl], tp)

                def emit_ph3(s0):
                    # fused Wo projection + output DMA for chunk s0's tokens
                    for j in range(4):
                        st = 4 * s0 + j
                        tsl = slice(128 * st, 128 * (st + 1))
                        y_sb = ph2.tile([128, C], F16, tag="y_sb", bufs=4)
                        for cc in range(2):
                            csl = slice(512 * cc, 512 * (cc + 1))
                            ps_y = accps.tile([128, 512], F32, tag="acc",
                                              name=f"psy{st}_{cc}")
                            for x in range(2):
                                nc.tensor.matmul(
                                    ps_y, att2[x][:, tsl], wo_sb[x][:, csl],
                                    start=(x == 0), stop=(x == 1))
                            nc.vector.tensor_copy(y_sb[:, csl], ps_y)
                        eng = nc.sync if st % 2 == 0 else nc.scalar
                        eng.dma_start(out=out.ap()[tsl, :], in_=y_sb)

                def emit_step(s0, tt, qk_alloc):
                    # QK + exp for (s0, tt); then the pending Wo projection
                    # (so its PSUM allocs precede this chunk's acc allocs in
                    # the accps rotation); then PV (and, at chunk wrap,
                    # normalize+transpose) for the PREVIOUS step.
                    ssl = slice(512 * s0, 512 * (s0 + 1))
                    tsl = slice(128 * tt, 128 * (tt + 1))
                    exps_g = []
                    for g in range(2):
                        qk = qk_alloc(g)
                        for hh in range(2):
                            po = 64 * hh
                            nc.tensor.matmul(
                                qk[:, 512 * hh:512 * (hh + 1)],
                                kri[g][po:po + 64, tsl],
                                qri[g][po:po + 64, ssl],
                                start=True, stop=True,
                                tile_position=(po, 0))
                        exps = ph2.tile([128, 1024], BF16, tag=f"ex{g}",
                                        bufs=4)
                        nc.scalar.activation(
                            exps, qk, mybir.ActivationFunctionType.Exp,
                            scale=SCALE)
                        exps_g.append(exps)
                    if st2["ph3"] is not None:
                        emit_ph3(st2["ph3"])
                        st2["ph3"] = None
                    if st2["pv"] is not None:
                        emit_pv(st2["pv"])
                        if st2["pv"][0] == TT - 1:
                            emit_fin_a(st2["pv"][1])
                            st2["ph3"] = st2["pv"][1]
                    st2["pv"] = (tt, s0, exps_g)

                with tc.tile_pool(name="ph1", bufs=2) as ph1, \
                     tc.tile_pool(name="ph1ps", bufs=4, space="PSUM") as ph1ps, \
                     tc.tile_pool(name="ropet", bufs=2) as ropet:
                    # DMA issue order = the phase-1 critical path: k weights,
                    # then chunk-0 x/freqs, then the remaining weights.
                    w_sb = {}
                    for nm, dram in (("kr", wkr), ("ki", wki)):
                        w = ph1.tile([128, KT * 128], BF16, name=f"w_{nm}",
                                     tag=nm, bufs=1)
                        nc.sync.dma_start(out=w, in_=dram.ap())
                        w_sb[nm] = w
                    xtqs, frs, fis = [], [], []
                    for s0 in range(SC):
                        sl = slice(512 * s0, 512 * (s0 + 1))
                        xtq = ph1.tile([128, KT, 512], BF16, tag="xtq",
                                       bufs=2)
                        nc.sync.dma_start(out=xtq, in_=xt_r[:, :, sl])
                        fr_sb = ph1.tile([128, 512], F32, tag="fr", bufs=2)
                        fi_sb = ph1.tile([128, 512], F32, tag="fi", bufs=2)
                        nc.sync.dma_start(out=fr_sb, in_=fr.ap()[:, sl])
                        nc.sync.dma_start(out=fi_sb, in_=fi.ap()[:, sl])
                        xtqs.append(xtq)
                        frs.append(fr_sb)
                        fis.append(fi_sb)
                        if s0 == 0:
                            for nm, dram in (("qr", wqr), ("qi", wqi)):
                                w = ph1.tile([128, KT * 128], BF16,
                                             name=f"w_{nm}", tag=nm, bufs=1)
                                nc.sync.dma_start(out=w, in_=dram.ap())
                                w_sb[nm] = w
                            wv_sb = ph1.tile([128, KT * 256], BF16, tag="wv",
                                             bufs=1)
                            nc.sync.dma_start(out=wv_sb, in_=wv.ap())
                            for x in range(2):
                                nc.sync.dma_start(out=wo_sb[x],
                                                  in_=wo.ap()[x])
                            nc.sync.dma_start(out=ident_sb, in_=ident.ap())
                            nc.vector.tensor_copy(ident32_sb, ident_sb)

                    def do_chunk(s0, steps=()):
                        steps = list(steps)
                        sl = slice(512 * s0, 512 * (s0 + 1))
                        xtq, fr_sb, fi_sb = xtqs[s0], frs[s0], fis[s0]
                        # k and q projections + rope for this token chunk
                        for nm, dsts in (("k", kri), ("q", qri)):
                            wr_, wi_ = w_sb[nm + "r"], w_sb[nm + "i"]
                            ps_r = ph1ps.tile([128, 512], F32, tag="proj")
                            ps_i = ph1ps.tile([128, 512], F32, tag="proj")
                            for kt in range(KT):
                                nc.tensor.matmul(
                                    ps_r, wr_[:, 128 * kt:128 * (kt + 1)],
                                    xtq[:, kt, :],
                                    start=(kt == 0), stop=(kt == KT - 1))
                            for kt in range(KT):
                                nc.tensor.matmul(
                                    ps_i, wi_[:, 128 * kt:128 * (kt + 1)],
                                    xtq[:, kt, :],
                                    start=(kt == 0), stop=(kt == KT - 1))
                            # rope: r' = r*fr - i*fi ; i' = r*fi + i*fr
                            t_rr = ropet.tile([128, 512], F32, tag="t0")
                            t_ii = ropet.tile([128, 512], F32, tag="t1")
                            t_ri = ropet.tile([128, 512], F32, tag="t2")
                            t_ir = ropet.tile([128, 512], F32, tag="t3")
                            nc.vector.tensor_tensor(t_rr, ps_r, fr_sb,
                                                    op=mybir.AluOpType.mult)
                            nc.vector.tensor_tensor(t_ii, ps_i, fi_sb,
                                                    op=mybir.AluOpType.mult)
                            nc.vector.tensor_tensor(t_ri, ps_r, fi_sb,
                                                    op=mybir.AluOpType.mult)
                            nc.vector.tensor_tensor(t_ir, ps_i, fr_sb,
                                                    op=mybir.AluOpType.mult)
                            ro = ropet.tile([128, 512], F32, tag="ro")
                            io = ropet.tile([128, 512], F32, tag="io")
                            nc.vector.tensor_tensor(
                                ro, t_rr, t_ii, op=mybir.AluOpType.subtract)
                            nc.vector.tensor_tensor(
                                io, t_ri, t_ir, op=mybir.AluOpType.add)
                            # repack into pair tiles, split Act/Pool engines
                            # (32-row copies: the packed layout interleaves
                            # re/im halves; strided partition APs don't
                            # compile)
                            for x in range(2):
                                dst = dsts[x][:, sl]
                                for hh in range(2):
                                    nc.scalar.copy(
                                        dst[64 * hh:64 * hh + 32, :],
                                        ro[64 * x + 32 * hh:
                                           64 * x + 32 * (hh + 1), :])
                                    nc.gpsimd.tensor_copy(
                                        dst[64 * hh + 32:64 * (hh + 1), :],
                                        io[64 * x + 32 * hh:
                                           64 * x + 32 * (hh + 1), :])
                            for th in steps[:2]:
                                th()
                            del steps[:2]

                        # v projection into [t, f] with ones cols interleaved
                        for tl in range(4):
                            tt = 4 * s0 + tl
                            ps_v = ph1ps.tile([128, 256], F32, tag="proj")
                            for kt in range(KT):
                                nc.tensor.matmul(
                                    ps_v, xtq[:, kt, 128 * tl:128 * (tl + 1)],
                                    wv_sb[:, 256 * kt:256 * (kt + 1)],
                                    start=(kt == 0), stop=(kt == KT - 1))
                            # strided evict: head h -> cols [65h, 65h+64)
                            vv = vaug_sb[:, tt, :].rearrange(
                                "p (h d) -> p h d", h=HL)
                            nc.scalar.copy(
                                vv[:, :, 0:64],
                                ps_v.rearrange("p (h d) -> p h d", h=HL))
                        for th in steps:
                            th()

                    for c in range(SC):
                        do_chunk(c)

                # ------------- attention for chunks 1..3 ----------------
                with tc.tile_pool(name="qkps", bufs=1,
                                  space="PSUM") as qkps:
                    def qk_alloc(g):
                        return qkps.tile([128, 1024], F32, tag=f"qk{g}",
                                         bufs=1, name=f"qk_{g}")
                    for i in range(SC * TT):
                        s0, tt = divmod(i, TT)
                        emit_step(s0, tt, qk_alloc)
                    emit_pv(st2["pv"])
                    emit_fin_a(st2["pv"][1])
                    emit_ph3(st2["pv"][1])

    nc.compile()
    return nc


def make_inputs(x, freqs, Wq, Wk, Wv, Wo):
    """Build the 8 per-core input maps."""
    rnd = lambda a: np.ascontiguousarray(a, dtype=ml_dtypes.bfloat16)  # noqa: E731

    # deinterleave permutations of the 256 local feature rows
    p = np.arange(128)
    real_rows = 64 * (p // 32) + 2 * (p % 32)       # within local 256 block
    imag_rows = real_rows + 1

    frh = np.ascontiguousarray(np.tile(freqs[:, :, 0].T, (HL, 1)),
                               dtype=np.float32)    # [128, S]
    fih = np.ascontiguousarray(np.tile(freqs[:, :, 1].T, (HL, 1)),
                               dtype=np.float32)

    def proj_weight(W, rows):
        # lhsT tiles: [128 c-part, KT*128], w[p, kt*128+m] = W[base+rows[m], kt*128+p]
        wt = W[rows, :]                              # [128, C]
        return rnd(wt.T.reshape(KT, 128, 128).transpose(1, 0, 2)
                   .reshape(128, KT * 128))

    in_maps = []
    for c in range(N_CORES):
        b, hg = divmod(c, 4)
        base = 256 * hg
        wqr = proj_weight(Wq, base + real_rows)
        wqi = proj_weight(Wq, base + imag_rows)
        wkr = proj_weight(Wk, base + real_rows)
        wki = proj_weight(Wk, base + imag_rows)
        # v: [128 c-part, KT*256], wv[p, kt*256+f] = Wv[base+f, kt*128+p]
        wvt = Wv[base:base + F, :].T                 # [C, F]
        wv_ = rnd(wvt.reshape(KT, 128, F).transpose(1, 0, 2)
                  .reshape(128, KT * F))
        # wo: [2, 128, C]; pair tile x rows = Wo columns for heads 2x,2x+1
        wo_ = np.empty((2, 128, C), np.float32)
        for xx in range(2):
            wo_[xx] = Wo[:, base + 128 * xx: base + 128 * (xx + 1)].T
        in_maps.append({
            "xt": rnd(x[b].T),
            "ident": rnd(np.eye(128, dtype=np.float32)),
            "wqr": wqr, "wqi": wqi, "wkr": wkr, "wki": wki,
            "wv": wv_, "wo": rnd(wo_),
            "fr": frh, "fi": fih,
        })
    return in_maps


def kernel(x, freqs, Wq, Wk, Wv, Wo):
    global _CACHED_NC
    x = np.asarray(x, dtype=np.float32)
    freqs = np.asarray(freqs, dtype=np.float32)
    Wq = np.asarray(Wq, dtype=np.float32)
    Wk = np.asarray(Wk, dtype=np.float32)
    Wv = np.asarray(Wv, dtype=np.float32)
    Wo = np.asarray(Wo, dtype=np.float32)

    in_maps = make_inputs(x, freqs, Wq, Wk, Wv, Wo)
    if _CACHED_NC is None:
        _CACHED_NC = build_module()
    res = run_bass_kernel_spmd(_CACHED_NC, in_maps,
                               core_ids=list(range(N_CORES)))
    outs = [np.asarray(r["out"], dtype=np.float32) for r in res.results]
    y = np.empty((B, S, C), np.float32)
    for b in range(B):
        y[b] = outs[4 * b] + outs[4 * b + 1] + outs[4 * b + 2] + outs[4 * b + 3]
    return y


if __name__ == "__main__":
    rng = np.random.default_rng(0)
    x = rng.standard_normal((B, S, C)).astype(np.float32)
    freqs = rng.standard_normal((S, DH, 2)).astype(np.float32)
    ws = [(rng.standard_normal((C, C)) * C ** -0.5).astype(np.float32)
          for _ in range(4)]
    y = kernel(x, freqs, *ws)
    print("out", y.shape, y.dtype, float(np.abs(y).mean()))


# revision 54
# speedup vs baseline: 1.0589x; 1.0503x over previous
"""Trainium2 Bass kernel for AttentionWithComplexRoPE.

Strategy (8 NeuronCores): data-parallel over batch (B=2) x tensor-parallel
over heads (16 heads -> 4 per core). Core c handles batch c//4, heads
[4*(c%4), 4*(c%4)+4).

Per-core pipeline (~224 us modeled vs 432 us baseline):
  phase 1: q/k projections (bf16 x/W, f32 PSUM); RoPE as wide DVE
           elementwise ops; results repacked (Act+Pool 32-row copies) into
           PAIR-PACKED tiles qri/kri[X] [128 = 2 heads x (re32|im32), t]
           so QK runs as ONE K=64 fp32r matmul per head. v in [t, f]
           layout with a ones column (softmax-denominator trick). DMA
           issue order puts k-weights and chunk-0 x first.
  phase 2+3 (fused): flat software pipeline over (query-chunk s0, key-tile
           tt). Per step: 4 K=64 QK matmuls -> [128,1024] PSUM x2 tags ->
           Exp on ScalarE (bf16 out, 4-deep buffers) -> 16 PV matmuls with
           QUERIES ON THE OUTPUT PARTITION DIM (lhsT = exp tile slice,
           moving = v_aug [128,65]; 65-row cost at full K=128/M=128). PV
           accumulates into 4 [128, 4hx65] acc tiles; col 64 of each head
           block is the softmax denominator -> a per-PARTITION scalar.
           NOTE: the PSUM `start` flag zeroes the whole BANK, so only the
           first region-write of an acc/transpose tile carries start=True.
           At each chunk wrap: normalize via DVE reciprocal +
           tensor_scalar_mul, PE-transpose (via identity) back into the
           [d, t] pair layout att2, then the Wo projection + f16 output
           DMA ride the same accps rotation (no separate phase-3 tail).
           The Act engine's exp stream (0.83 ns/col, B*H*S^2/8 cols/core)
           is the kernel's hard floor.
Host: slice/permute weights (bf16), transpose x (bf16), replicate freqs
(f32), identity for PE transpose; sum the 4 per-batch f16 partials in f32.

PSUM budget: ph1 proj(4) | qk tags(2x2) + acc/transpose/Wo rotation(4).
"""
import sys

if "/opt/trn_rl_repo" not in sys.path:
    sys.path.insert(0, "/opt/trn_rl_repo")

import ml_dtypes
import numpy as np

import concourse.bass as bass
import concourse.mybir as mybir
import concourse.tile as tile
from concourse import bacc
from concourse.bass_utils import run_bass_kernel_spmd

F32 = mybir.dt.float32
F32R = mybir.dt.float32r
BF16 = mybir.dt.bfloat16
F16 = mybir.dt.float16

B, S, C = 2, 2048, 1024
H = 16                      # global heads
HL = 4                      # heads per core
D = C // H                  # 64
DH = 32                     # complex pairs per head
F = HL * D                  # 256 local features
N_CORES = 8
KT = C // 128               # 8 contraction tiles for projections
TT = S // 128               # 16 token tiles
SC = S // 512               # 4 s-chunks
SCALE = float(D) ** -0.5

_CACHED_NC = None


def build_module():
    nc = bacc.Bacc("TRN2", target_bir_lowering=False)

    xt = nc.dram_tensor("xt", [C, S], BF16, kind="ExternalInput")
    wqr = nc.dram_tensor("wqr", [128, KT * 128], BF16, kind="ExternalInput")
    wqi = nc.dram_tensor("wqi", [128, KT * 128], BF16, kind="ExternalInput")
    wkr = nc.dram_tensor("wkr", [128, KT * 128], BF16, kind="ExternalInput")
    wki = nc.dram_tensor("wki", [128, KT * 128], BF16, kind="ExternalInput")
    wv = nc.dram_tensor("wv", [128, KT * 256], BF16, kind="ExternalInput")
    wo = nc.dram_tensor("wo", [2, 128, C], BF16, kind="ExternalInput")
    fr = nc.dram_tensor("fr", [128, S], F32, kind="ExternalInput")
    fi = nc.dram_tensor("fi", [128, S], F32, kind="ExternalInput")
    ident = nc.dram_tensor("ident", [128, 128], BF16, kind="ExternalInput")
    out = nc.dram_tensor("out", [S, C], F16, kind="ExternalOutput")

    with tile.TileContext(nc) as tc:
        with tc.tile_pool(name="persist", bufs=1) as persist:
            # persistent sbuf tensors
            qri = [persist.tile([128, S], F32R, name=f"qri{x}")
                   for x in range(2)]
            kri = [persist.tile([128, S], F32R, name=f"kri{x}")
                   for x in range(2)]
            vaug_sb = persist.tile([128, TT, HL * 65], BF16)  # v + ones col
            ident_sb = persist.tile([128, 128], BF16)
            ident32_sb = persist.tile([128, 128], F32)
            att2 = [persist.tile([128, S], BF16, name=f"att2_{x}")
                    for x in range(2)]
            wo_sb = [persist.tile([128, C], BF16, name=f"wo{x}_sb")
                     for x in range(2)]

            # ones columns of v_aug (col 64 of each head block), all at once
            nc.vector.memset(
                vaug_sb.rearrange("p tt (h d) -> p tt h d", h=HL)
                [:, :, :, 64:65], 1.0)

            # ---------------- phase 1: projections + rope ----------------
            xt_r = xt.rearrange("(kt p) s -> p kt s", p=128)

            # Attention pools span phase 1 so chunk-0 attention (s0=0) can
            # interleave with the remaining projection chunks. PSUM budget
            # during the overlap: ph1ps(2) + qke(2) + accps(4) = 8 banks.
            with tc.tile_pool(name="ph2", bufs=4) as ph2, \
                 tc.tile_pool(name="ph2s", bufs=4) as ph2s, \
                 tc.tile_pool(name="accps", bufs=4, space="PSUM") as accps:
                st2 = {"accs": None, "pv": None, "ph3": None}

                def emit_pv(pend):
                    # PV with queries on the output PARTITION dim: lhsT is
                    # the exp tile slice [128 keys, 128 queries], moving is
                    # v_aug [128 keys, 65] -> 65-row matmuls at full K=128,
                    # M=128 PE utilization (bf16). acc[qt] = [128 q, 4h x 65]
                    # accumulated over key tiles; col 64 of each head block
                    # is the softmax denominator (a per-PARTITION scalar).
                    ptt, ps0, pexps = pend
                    if ptt == 0:
                        st2["accs"] = [accps.tile([128, HL * 65], F32,
                                                  tag="acc",
                                                  name=f"acc{ps0}_{qt}")
                                       for qt in range(4)]
                    paccs = st2["accs"]
                    for qt in range(4):
                        for h in range(HL):
                            po = 512 * (h % 2) + 128 * qt
                            nc.tensor.matmul(
                                paccs[qt][:, 65 * h:65 * (h + 1)],
                                pexps[h // 2][:, po:po + 128],
                                vaug_sb[:, ptt, 65 * h:65 * (h + 1)],
                                start=(ptt == 0 and h == 0),
                                stop=(ptt == TT - 1),
                                skip_group_check=True)

                def emit_fin_a(s0):
                    # softmax normalize (per-partition reciprocals) and PE
                    # transpose back into the [d, t] pair layout att2.
                    paccs = st2["accs"]
                    ssl = slice(512 * s0, 512 * (s0 + 1))
                    att_qts = []
                    for qt in range(4):
                        acc = paccs[qt]
                        accv = acc.rearrange("p (h e) -> p h e", h=HL)
                        recip4 = ph2s.tile([128, HL], F32, tag="recip")
                        nc.vector.reciprocal(recip4, accv[:, :, 64])
                        att_qt = ph2s.tile([128, F], F32, tag="attq",
                                           bufs=4)
                        for h in range(HL):
                            nc.vector.tensor_scalar_mul(
                                att_qt[:, 64 * h:64 * (h + 1)],
                                acc[:, 65 * h:65 * h + 64],
                                recip4[:, h:h + 1])
                        att_qts.append(att_qt)
                    for x in range(2):
                        tp = accps.tile([128, 512], F32, tag="acc",
                                        name=f"tp{x}")
                        for qt in range(4):
                            nc.tensor.matmul(
                                tp[:, 128 * qt:128 * (qt + 1)],
                                att_qts[qt][:, 128 * x:128 * (x + 1)],
                                ident32_sb, is_transpose=True,
                                start=(qt == 0), stop=(qt == 3),
                                skip_group_check=True)
                        nc.vector.tensor_copy(att2[x][:, ssl], tp)

                def emit_ph3(s0):
                    # fused Wo projection + output DMA for chunk s0's tokens
                    for j in range(4):
                        st = 4 * s0 + j
                        tsl = slice(128 * st, 128 * (st + 1))
                        y_sb = ph2.tile([128, C], F16, tag="y_sb", bufs=4)
                        for cc in range(2):
                            csl = slice(512 * cc, 512 * (cc + 1))
                            ps_y = accps.tile([128, 512], F32, tag="acc",
                                              name=f"psy{st}_{cc}")
                            for x in range(2):
                                nc.tensor.matmul(
                                    ps_y, att2[x][:, tsl], wo_sb[x][:, csl],
                                    start=(x == 0), stop=(x == 1))
                            nc.vector.tensor_copy(y_sb[:, csl], ps_y)
                        eng = nc.sync if st % 2 == 0 else nc.scalar
                        eng.dma_start(out=out.ap()[tsl, :], in_=y_sb)

                def emit_step(s0, tt, qk_alloc):
                    # QK + exp for (s0, tt); then the pending Wo projection
                    # (so its PSUM allocs precede this chunk's acc allocs in
                    # the accps rotation); then PV (and, at chunk wrap,
                    # normalize+transpose) for the PREVIOUS step.
                    ssl = slice(512 * s0, 512 * (s0 + 1))
                    tsl = slice(128 * tt, 128 * (tt + 1))
                    exps_g = []
                    for g in range(2):
                        qk = qk_alloc(g)
                        for hh in range(2):
                            po = 64 * hh
                            nc.tensor.matmul(
                                qk[:, 512 * hh:512 * (hh + 1)],
                                kri[g][po:po + 64, tsl],
                                qri[g][po:po + 64, ssl],
                                start=True, stop=True,
                                tile_position=(po, 0))
                        exps = ph2.tile([128, 1024], BF16, tag=f"ex{g}",
                                        bufs=4)
                        nc.scalar.activation(
                            exps, qk, mybir.ActivationFunctionType.Exp,
                            scale=SCALE)
                        exps_g.append(exps)
                    if st2["ph3"] is not None:
                        emit_ph3(st2["ph3"])
                        st2["ph3"] = None
                    if st2["pv"] is not None:
                        emit_pv(st2["pv"])
                        if st2["pv"][0] == TT - 1:
                            emit_fin_a(st2["pv"][1])
                            st2["ph3"] = st2["pv"][1]
                    st2["pv"] = (tt, s0, exps_g)

                with tc.tile_pool(name="ph1", bufs=2) as ph1, \
                     tc.tile_pool(name="ph1ps", bufs=4, space="PSUM") as ph1ps, \
                     tc.tile_pool(name="ropet", bufs=2) as ropet:
                    # DMA issue order = the phase-1 critical path: k weights,
                    # then chunk-0 x/freqs, then the remaining weights.
                    w_sb = {}
                    for nm, dram in (("kr", wkr), ("ki", wki)):
                        w = ph1.tile([128, KT * 128], BF16, name=f"w_{nm}",
                                     tag=nm, bufs=1)
                        nc.sync.dma_start(out=w, in_=dram.ap())
                        w_sb[nm] = w
                    xtqs, frs, fis = [], [], []
                    for s0 in range(SC):
                        sl = slice(512 * s0, 512 * (s0 + 1))
                        xtq = ph1.tile([128, KT, 512], BF16, tag="xtq",
                                       bufs=2)
                        if s0 == 0:
                            # split chunk-0's load so the first k-projection
                            # K-chain starts after only the kt=0-1 piece
                            nc.sync.dma_start(out=xtq[:, 0:2, :],
                                              in_=xt_r[:, 0:2, sl])
                            nc.sync.dma_start(out=xtq[:, 2:KT, :],
                                              in_=xt_r[:, 2:KT, sl])
                        else:
                            nc.sync.dma_start(out=xtq, in_=xt_r[:, :, sl])
                        fr_sb = ph1.tile([128, 512], F32, tag="fr", bufs=2)
                        fi_sb = ph1.tile([128, 512], F32, tag="fi", bufs=2)
                        nc.sync.dma_start(out=fr_sb, in_=fr.ap()[:, sl])
                        nc.sync.dma_start(out=fi_sb, in_=fi.ap()[:, sl])
                        xtqs.append(xtq)
                        frs.append(fr_sb)
                        fis.append(fi_sb)
                        if s0 == 0:
                            for nm, dram in (("qr", wqr), ("qi", wqi)):
                                w = ph1.tile([128, KT * 128], BF16,
                                             name=f"w_{nm}", tag=nm, bufs=1)
                                nc.sync.dma_start(out=w, in_=dram.ap())
                                w_sb[nm] = w
                            wv_sb = ph1.tile([128, KT * 256], BF16, tag="wv",
                                             bufs=1)
                            nc.sync.dma_start(out=wv_sb, in_=wv.ap())
                            for x in range(2):
                                nc.sync.dma_start(out=wo_sb[x],
                                                  in_=wo.ap()[x])
                            nc.sync.dma_start(out=ident_sb, in_=ident.ap())
                            nc.vector.tensor_copy(ident32_sb, ident_sb)

                    def do_chunk(s0, steps=()):
                        steps = list(steps)
                        sl = slice(512 * s0, 512 * (s0 + 1))
                        xtq, fr_sb, fi_sb = xtqs[s0], frs[s0], fis[s0]
                        # k and q projections + rope for this token chunk
                        for nm, dsts in (("k", kri), ("q", qri)):
                            wr_, wi_ = w_sb[nm + "r"], w_sb[nm + "i"]
                            ps_r = ph1ps.tile([128, 512], F32, tag="proj")
                            ps_i = ph1ps.tile([128, 512], F32, tag="proj")
                            for kt in range(KT):
                                nc.tensor.matmul(
                                    ps_r, wr_[:, 128 * kt:128 * (kt + 1)],
                                    xtq[:, kt, :],
                                    start=(kt == 0), stop=(kt == KT - 1))
                            for kt in range(KT):
                                nc.tensor.matmul(
                                    ps_i, wi_[:, 128 * kt:128 * (kt + 1)],
                                    xtq[:, kt, :],
                                    start=(kt == 0), stop=(kt == KT - 1))
                            # rope: r' = r*fr - i*fi ; i' = r*fi + i*fr
                            t_rr = ropet.tile([128, 512], F32, tag="t0")
                            t_ii = ropet.tile([128, 512], F32, tag="t1")
                            t_ri = ropet.tile([128, 512], F32, tag="t2")
                            t_ir = ropet.tile([128, 512], F32, tag="t3")
                            nc.vector.tensor_tensor(t_rr, ps_r, fr_sb,
                                                    op=mybir.AluOpType.mult)
                            nc.vector.tensor_tensor(t_ii, ps_i, fi_sb,
                                                    op=mybir.AluOpType.mult)
                            nc.vector.tensor_tensor(t_ri, ps_r, fi_sb,
                                                    op=mybir.AluOpType.mult)
                            nc.vector.tensor_tensor(t_ir, ps_i, fr_sb,
                                                    op=mybir.AluOpType.mult)
                            ro = ropet.tile([128, 512], F32, tag="ro")
                            io = ropet.tile([128, 512], F32, tag="io")
                            nc.vector.tensor_tensor(
                                ro, t_rr, t_ii, op=mybir.AluOpType.subtract)
                            nc.vector.tensor_tensor(
                                io, t_ri, t_ir, op=mybir.AluOpType.add)
                            # repack into pair tiles, split Act/Pool engines
                            # (32-row copies: the packed layout interleaves
                            # re/im halves; strided partition APs don't
                            # compile)
                            for x in range(2):
                                dst = dsts[x][:, sl]
                                for hh in range(2):
                                    nc.scalar.copy(
                                        dst[64 * hh:64 * hh + 32, :],
                                        ro[64 * x + 32 * hh:
                                           64 * x + 32 * (hh + 1), :])
                                    nc.gpsimd.tensor_copy(
                                        dst[64 * hh + 32:64 * (hh + 1), :],
                                        io[64 * x + 32 * hh:
                                           64 * x + 32 * (hh + 1), :])
                            for th in steps[:2]:
                                th()
                            del steps[:2]

                        # v projection into [t, f] with ones cols interleaved
                        for tl in range(4):
                            tt = 4 * s0 + tl
                            ps_v = ph1ps.tile([128, 256], F32, tag="proj")
                            for kt in range(KT):
                                nc.tensor.matmul(
                                    ps_v, xtq[:, kt, 128 * tl:128 * (tl + 1)],
                                    wv_sb[:, 256 * kt:256 * (kt + 1)],
                                    start=(kt == 0), stop=(kt == KT - 1))
                            # strided evict: head h -> cols [65h, 65h+64)
                            vv = vaug_sb[:, tt, :].rearrange(
                                "p (h d) -> p h d", h=HL)
                            nc.scalar.copy(
                                vv[:, :, 0:64],
                                ps_v.rearrange("p (h d) -> p h d", h=HL))
                        for th in steps:
                            th()

                    for c in range(SC):
                        do_chunk(c)

                # ------------- attention for chunks 1..3 ----------------
                with tc.tile_pool(name="qkps", bufs=1,
                                  space="PSUM") as qkps:
                    def qk_alloc(g):
                        return qkps.tile([128, 1024], F32, tag=f"qk{g}",
                                         bufs=1, name=f"qk_{g}")
                    for i in range(SC * TT):
                        s0, tt = divmod(i, TT)
                        emit_step(s0, tt, qk_alloc)
                    emit_pv(st2["pv"])
                    emit_fin_a(st2["pv"][1])
                    emit_ph3(st2["pv"][1])

    nc.compile()
    return nc


def make_inputs(x, freqs, Wq, Wk, Wv, Wo):
    """Build the 8 per-core input maps."""
    rnd = lambda a: np.ascontiguousarray(a, dtype=ml_dtypes.bfloat16)  # noqa: E731

    # deinterleave permutations of the 256 local feature rows
    p = np.arange(128)
    real_rows = 64 * (p // 32) + 2 * (p % 32)       # within local 256 block
    imag_rows = real_rows + 1

    frh = np.ascontiguousarray(np.tile(freqs[:, :, 0].T, (HL, 1)),
                               dtype=np.float32)    # [128, S]
    fih = np.ascontiguousarray(np.tile(freqs[:, :, 1].T, (HL, 1)),
                               dtype=np.float32)

    def proj_weight(W, rows):
        # lhsT tiles: [128 c-part, KT*128], w[p, kt*128+m] = W[base+rows[m], kt*128+p]
        wt = W[rows, :]                              # [128, C]
        return rnd(wt.T.reshape(KT, 128, 128).transpose(1, 0, 2)
                   .reshape(128, KT * 128))

    in_maps = []
    for c in range(N_CORES):
        b, hg = divmod(c, 4)
        base = 256 * hg
        wqr = proj_weight(Wq, base + real_rows)
        wqi = proj_weight(Wq, base + imag_rows)
        wkr = proj_weight(Wk, base + real_rows)
        wki = proj_weight(Wk, base + imag_rows)
        # v: [128 c-part, KT*256], wv[p, kt*256+f] = Wv[base+f, kt*128+p]
        wvt = Wv[base:base + F, :].T                 # [C, F]
        wv_ = rnd(wvt.reshape(KT, 128, F).transpose(1, 0, 2)
                  .reshape(128, KT * F))
        # wo: [2, 128, C]; pair tile x rows = Wo columns for heads 2x,2x+1
        wo_ = np.empty((2, 128, C), np.float32)
        for xx in range(2):
            wo_[xx] = Wo[:, base + 128 * xx: base + 128 * (xx + 1)].T
        in_maps.append({
            "xt": rnd(x[b].T),
            "ident": rnd(np.eye(128, dtype=np.float32)),
            "wqr": wqr, "wqi": wqi, "wkr": wkr, "wki": wki,
            "wv": wv_, "wo": rnd(wo_),
            "fr": frh, "fi": fih,
        })
    return in_maps


def kernel(x, freqs, Wq, Wk, Wv, Wo):
    global _CACHED_NC
    x = np.asarray(x, dtype=np.float32)
    freqs = np.asarray(freqs, dtype=np.float32)
    Wq = np.asarray(Wq, dtype=np.float32)
    Wk = np.asarray(Wk, dtype=np.float32)
    Wv = np.asarray(Wv, dtype=np.float32)
    Wo = np.asarray(Wo, dtype=np.float32)

    in_maps = make_inputs(x, freqs, Wq, Wk, Wv, Wo)
    if _CACHED_NC is None:
        _CACHED_NC = build_module()
    res = run_bass_kernel_spmd(_CACHED_NC, in_maps,
                               core_ids=list(range(N_CORES)))
    outs = [np.asarray(r["out"], dtype=np.float32) for r in res.results]
    y = np.empty((B, S, C), np.float32)
    for b in range(B):
        y[b] = outs[4 * b] + outs[4 * b + 1] + outs[4 * b + 2] + outs[4 * b + 3]
    return y


if __name__ == "__main__":
    rng = np.random.default_rng(0)
    x = rng.standard_normal((B, S, C)).astype(np.float32)
    freqs = rng.standard_normal((S, DH, 2)).astype(np.float32)
    ws = [(rng.standard_normal((C, C)) * C ** -0.5).astype(np.float32)
          for _ in range(4)]
    y = kernel(x, freqs, *ws)
    print("out", y.shape, y.dtype, float(np.abs(y).mean()))


# revision 58
# speedup vs baseline: 1.0597x; 1.0007x over previous
"""Trainium2 Bass kernel for AttentionWithComplexRoPE.

Strategy (8 NeuronCores): data-parallel over batch (B=2) x tensor-parallel
over heads (16 heads -> 4 per core). Core c handles batch c//4, heads
[4*(c%4), 4*(c%4)+4).

Per-core pipeline (~213 us modeled vs 432 us baseline):
  phase 1: q/k projections (bf16 x/W, f32 PSUM); RoPE as wide DVE
           elementwise ops; results repacked (Act+Pool 32-row copies) into
           PAIR-PACKED tiles qri/kri[X] [128 = 2 heads x (re32|im32), t]
           so QK runs as ONE K=64 fp32r matmul per head. v in [t, f]
           layout with a ones column (softmax-denominator trick). DMA
           issue order puts k-weights and chunk-0 x first.
  phase 2+3 (fused): flat software pipeline over (query-chunk s0, key-tile
           tt). Per step: 4 K=64 QK matmuls -> [128,1024] PSUM x2 tags ->
           Exp on ScalarE (bf16 out, 4-deep buffers) -> 16 PV matmuls with
           QUERIES ON THE OUTPUT PARTITION DIM (lhsT = exp tile slice,
           moving = v_aug [128,65]; 65-row cost at full K=128/M=128). PV
           accumulates into 4 [128, 4hx65] acc tiles; col 64 of each head
           block is the softmax denominator -> a per-PARTITION scalar.
           NOTE: the PSUM `start` flag zeroes the whole BANK, so only the
           first region-write of an acc/transpose tile carries start=True.
           At each chunk wrap: normalize via DVE reciprocal +
           tensor_scalar_mul, PE-transpose (via identity) back into the
           [d, t] pair layout att2, then the Wo projection + f16 output
           DMA ride the same accps rotation (no separate phase-3 tail).
           The Act engine's exp stream (0.83 ns/col, B*H*S^2/8 cols/core)
           is the kernel's hard floor.
Host: slice/permute weights (bf16), transpose x (bf16), replicate freqs
(f32), identity for PE transpose; sum the 4 per-batch f16 partials in f32.

PSUM budget: ph1 proj(4) | qk tags(2x2) + acc/transpose/Wo rotation(4).
"""
import sys

if "/opt/trn_rl_repo" not in sys.path:
    sys.path.insert(0, "/opt/trn_rl_repo")

import ml_dtypes
import numpy as np

import concourse.bass as bass
import concourse.mybir as mybir
import concourse.tile as tile
from concourse import bacc
from concourse.bass_utils import run_bass_kernel_spmd

F32 = mybir.dt.float32
F32R = mybir.dt.float32r
BF16 = mybir.dt.bfloat16
F16 = mybir.dt.float16

B, S, C = 2, 2048, 1024
H = 16                      # global heads
HL = 4                      # heads per core
D = C // H                  # 64
DH = 32                     # complex pairs per head
F = HL * D                  # 256 local features
N_CORES = 8
KT = C // 128               # 8 contraction tiles for projections
TT = S // 128               # 16 token tiles
SC = S // 512               # 4 s-chunks
SCALE = float(D) ** -0.5

_CACHED_NC = None


def build_module():
    nc = bacc.Bacc("TRN2", target_bir_lowering=False)

    xt = nc.dram_tensor("xt", [C, S], BF16, kind="ExternalInput")
    wqr = nc.dram_tensor("wqr", [128, KT * 128], BF16, kind="ExternalInput")
    wqi = nc.dram_tensor("wqi", [128, KT * 128], BF16, kind="ExternalInput")
    wkr = nc.dram_tensor("wkr", [128, KT * 128], BF16, kind="ExternalInput")
    wki = nc.dram_tensor("wki", [128, KT * 128], BF16, kind="ExternalInput")
    wv = nc.dram_tensor("wv", [128, KT * 256], BF16, kind="ExternalInput")
    wo = nc.dram_tensor("wo", [2, 128, C], BF16, kind="ExternalInput")
    fr = nc.dram_tensor("fr", [128, S], F32, kind="ExternalInput")
    fi = nc.dram_tensor("fi", [128, S], F32, kind="ExternalInput")
    ident = nc.dram_tensor("ident", [128, 128], BF16, kind="ExternalInput")
    out = nc.dram_tensor("out", [S, C], F16, kind="ExternalOutput")

    with tile.TileContext(nc) as tc:
        with tc.tile_pool(name="persist", bufs=1) as persist:
            # persistent sbuf tensors
            qri = [persist.tile([128, S], F32R, name=f"qri{x}")
                   for x in range(2)]
            kri = [persist.tile([128, S], F32R, name=f"kri{x}")
                   for x in range(2)]
            vaug_sb = persist.tile([128, TT, HL * 65], BF16)  # v + ones col
            ident_sb = persist.tile([128, 128], BF16)
            ident32_sb = persist.tile([128, 128], F32)
            att2 = [persist.tile([128, S], BF16, name=f"att2_{x}")
                    for x in range(2)]
            wo_sb = [persist.tile([128, C], BF16, name=f"wo{x}_sb")
                     for x in range(2)]

            # ones columns of v_aug (col 64 of each head block), all at once
            nc.vector.memset(
                vaug_sb.rearrange("p tt (h d) -> p tt h d", h=HL)
                [:, :, :, 64:65], 1.0)

            # ---------------- phase 1: projections + rope ----------------
            xt_r = xt.rearrange("(kt p) s -> p kt s", p=128)

            # Attention pools span phase 1 so chunk-0 attention (s0=0) can
            # interleave with the remaining projection chunks. PSUM budget
            # during the overlap: ph1ps(2) + qke(2) + accps(4) = 8 banks.
            with tc.tile_pool(name="ph2", bufs=4) as ph2, \
                 tc.tile_pool(name="ph2s", bufs=4) as ph2s, \
                 tc.tile_pool(name="accps", bufs=4, space="PSUM") as accps:
                st2 = {"accs": None, "pv": None, "ph3": None}

                def emit_pv(pend):
                    # PV with queries on the output PARTITION dim: lhsT is
                    # the exp tile slice [128 keys, 128 queries], moving is
                    # v_aug [128 keys, 65] -> 65-row matmuls at full K=128,
                    # M=128 PE utilization (bf16). acc[qt] = [128 q, 4h x 65]
                    # accumulated over key tiles; col 64 of each head block
                    # is the softmax denominator (a per-PARTITION scalar).
                    ptt, ps0, pexps = pend
                    if ptt == 0:
                        st2["accs"] = [accps.tile([128, HL * 65], F32,
                                                  tag="acc",
                                                  name=f"acc{ps0}_{qt}")
                                       for qt in range(4)]
                    paccs = st2["accs"]
                    for qt in range(4):
                        for h in range(HL):
                            po = 512 * (h % 2) + 128 * qt
                            nc.tensor.matmul(
                                paccs[qt][:, 65 * h:65 * (h + 1)],
                                pexps[h // 2][:, po:po + 128],
                                vaug_sb[:, ptt, 65 * h:65 * (h + 1)],
                                start=(ptt == 0 and h == 0),
                                stop=(ptt == TT - 1),
                                skip_group_check=True)

                def emit_fin_a(s0):
                    # softmax normalize (per-partition reciprocals) and PE
                    # transpose back into the [d, t] pair layout att2.
                    paccs = st2["accs"]
                    ssl = slice(512 * s0, 512 * (s0 + 1))
                    att_qts = []
                    for qt in range(4):
                        acc = paccs[qt]
                        accv = acc.rearrange("p (h e) -> p h e", h=HL)
                        recip4 = ph2s.tile([128, HL], F32, tag="recip")
                        nc.vector.reciprocal(recip4, accv[:, :, 64])
                        att_qt = ph2s.tile([128, F], F32, tag="attq",
                                           bufs=4)
                        for h in range(HL):
                            nc.vector.tensor_scalar_mul(
                                att_qt[:, 64 * h:64 * (h + 1)],
                                acc[:, 65 * h:65 * h + 64],
                                recip4[:, h:h + 1])
                        att_qts.append(att_qt)
                    for x in range(2):
                        tp = accps.tile([128, 512], F32, tag="acc",
                                        name=f"tp{x}")
                        for qt in range(4):
                            nc.tensor.matmul(
                                tp[:, 128 * qt:128 * (qt + 1)],
                                att_qts[qt][:, 128 * x:128 * (x + 1)],
                                ident32_sb, is_transpose=True,
                                start=(qt == 0), stop=(qt == 3),
                                skip_group_check=True)
                        nc.vector.tensor_copy(att2[x][:, ssl], tp)

                def emit_ph3(s0):
                    # fused Wo projection + output DMA for chunk s0's tokens
                    for j in range(4):
                        st = 4 * s0 + j
                        tsl = slice(128 * st, 128 * (st + 1))
                        y_sb = ph2.tile([128, C], F16, tag="y_sb", bufs=4)
                        for cc in range(2):
                            csl = slice(512 * cc, 512 * (cc + 1))
                            ps_y = accps.tile([128, 512], F32, tag="acc",
                                              name=f"psy{st}_{cc}")
                            for x in range(2):
                                nc.tensor.matmul(
                                    ps_y, att2[x][:, tsl], wo_sb[x][:, csl],
                                    start=(x == 0), stop=(x == 1))
                            nc.vector.tensor_copy(y_sb[:, csl], ps_y)
                        eng = nc.sync if st % 2 == 0 else nc.scalar
                        eng.dma_start(out=out.ap()[tsl, :], in_=y_sb)

                def emit_step(s0, tt, qk_alloc):
                    # QK + exp for (s0, tt); then the pending Wo projection
                    # (so its PSUM allocs precede this chunk's acc allocs in
                    # the accps rotation); then PV (and, at chunk wrap,
                    # normalize+transpose) for the PREVIOUS step.
                    ssl = slice(512 * s0, 512 * (s0 + 1))
                    tsl = slice(128 * tt, 128 * (tt + 1))
                    exps_g = []
                    for g in range(2):
                        qk = qk_alloc(g)
                        for hh in range(2):
                            po = 64 * hh
                            nc.tensor.matmul(
                                qk[:, 512 * hh:512 * (hh + 1)],
                                kri[g][po:po + 64, tsl],
                                qri[g][po:po + 64, ssl],
                                start=True, stop=True,
                                tile_position=(po, 0))
                        exps = ph2.tile([128, 1024], BF16, tag=f"ex{g}",
                                        bufs=4)
                        nc.scalar.activation(
                            exps, qk, mybir.ActivationFunctionType.Exp,
                            scale=SCALE)
                        exps_g.append(exps)
                    if st2["ph3"] is not None:
                        emit_ph3(st2["ph3"])
                        st2["ph3"] = None
                    if st2["pv"] is not None:
                        emit_pv(st2["pv"])
                        if st2["pv"][0] == TT - 1:
                            emit_fin_a(st2["pv"][1])
                            st2["ph3"] = st2["pv"][1]
                    st2["pv"] = (tt, s0, exps_g)

                with tc.tile_pool(name="ph1", bufs=2) as ph1, \
                     tc.tile_pool(name="ph1ps", bufs=4, space="PSUM") as ph1ps, \
                     tc.tile_pool(name="ropet", bufs=2) as ropet:
                    # DMA issue order = the phase-1 critical path: k weights,
                    # then chunk-0 x/freqs, then the remaining weights.
                    w_sb = {}
                    for nm, dram in (("kr", wkr), ("ki", wki)):
                        w = ph1.tile([128, KT * 128], BF16, name=f"w_{nm}",
                                     tag=nm, bufs=1)
                        nc.sync.dma_start(out=w, in_=dram.ap())
                        w_sb[nm] = w
                    xtqs, frs, fis = [], [], []
                    for s0 in range(SC):
                        sl = slice(512 * s0, 512 * (s0 + 1))
                        xtq = ph1.tile([128, KT, 512], BF16, tag="xtq",
                                       bufs=2)
                        # split loads so each chunk's first k-projection
                        # K-chain starts after only the kt=0-1 piece
                        nc.sync.dma_start(out=xtq[:, 0:2, :],
                                          in_=xt_r[:, 0:2, sl])
                        nc.sync.dma_start(out=xtq[:, 2:KT, :],
                                          in_=xt_r[:, 2:KT, sl])
                        fr_sb = ph1.tile([128, 512], F32, tag="fr", bufs=2)
                        fi_sb = ph1.tile([128, 512], F32, tag="fi", bufs=2)
                        nc.sync.dma_start(out=fr_sb, in_=fr.ap()[:, sl])
                        nc.sync.dma_start(out=fi_sb, in_=fi.ap()[:, sl])
                        xtqs.append(xtq)
                        frs.append(fr_sb)
                        fis.append(fi_sb)
                        if s0 == 0:
                            for nm, dram in (("qr", wqr), ("qi", wqi)):
                                w = ph1.tile([128, KT * 128], BF16,
                                             name=f"w_{nm}", tag=nm, bufs=1)
                                nc.sync.dma_start(out=w, in_=dram.ap())
                                w_sb[nm] = w
                            wv_sb = ph1.tile([128, KT * 256], BF16, tag="wv",
                                             bufs=1)
                            nc.sync.dma_start(out=wv_sb, in_=wv.ap())
                            for x in range(2):
                                nc.sync.dma_start(out=wo_sb[x],
                                                  in_=wo.ap()[x])
                            nc.sync.dma_start(out=ident_sb, in_=ident.ap())
                            nc.vector.tensor_copy(ident32_sb, ident_sb)

                    def do_chunk(s0, steps=()):
                        steps = list(steps)
                        sl = slice(512 * s0, 512 * (s0 + 1))
                        xtq, fr_sb, fi_sb = xtqs[s0], frs[s0], fis[s0]
                        # k and q projections + rope for this token chunk
                        for nm, dsts in (("k", kri), ("q", qri)):
                            wr_, wi_ = w_sb[nm + "r"], w_sb[nm + "i"]
                            ps_r = ph1ps.tile([128, 512], F32, tag="proj")
                            ps_i = ph1ps.tile([128, 512], F32, tag="proj")
                            for kt in range(KT):
                                nc.tensor.matmul(
                                    ps_r, wr_[:, 128 * kt:128 * (kt + 1)],
                                    xtq[:, kt, :],
                                    start=(kt == 0), stop=(kt == KT - 1))
                            for kt in range(KT):
                                nc.tensor.matmul(
                                    ps_i, wi_[:, 128 * kt:128 * (kt + 1)],
                                    xtq[:, kt, :],
                                    start=(kt == 0), stop=(kt == KT - 1))
                            # rope: r' = r*fr - i*fi ; i' = r*fi + i*fr
                            t_rr = ropet.tile([128, 512], F32, tag="t0")
                            t_ii = ropet.tile([128, 512], F32, tag="t1")
                            t_ri = ropet.tile([128, 512], F32, tag="t2")
                            t_ir = ropet.tile([128, 512], F32, tag="t3")
                            nc.vector.tensor_tensor(t_rr, ps_r, fr_sb,
                                                    op=mybir.AluOpType.mult)
                            nc.vector.tensor_tensor(t_ii, ps_i, fi_sb,
                                                    op=mybir.AluOpType.mult)
                            nc.vector.tensor_tensor(t_ri, ps_r, fi_sb,
                                                    op=mybir.AluOpType.mult)
                            nc.vector.tensor_tensor(t_ir, ps_i, fr_sb,
                                                    op=mybir.AluOpType.mult)
                            ro = ropet.tile([128, 512], F32, tag="ro")
                            io = ropet.tile([128, 512], F32, tag="io")
                            nc.vector.tensor_tensor(
                                ro, t_rr, t_ii, op=mybir.AluOpType.subtract)
                            nc.vector.tensor_tensor(
                                io, t_ri, t_ir, op=mybir.AluOpType.add)
                            # repack into pair tiles, split Act/Pool engines
                            # (32-row copies: the packed layout interleaves
                            # re/im halves; strided partition APs don't
                            # compile)
                            for x in range(2):
                                dst = dsts[x][:, sl]
                                for hh in range(2):
                                    nc.scalar.copy(
                                        dst[64 * hh:64 * hh + 32, :],
                                        ro[64 * x + 32 * hh:
                                           64 * x + 32 * (hh + 1), :])
                                    nc.gpsimd.tensor_copy(
                                        dst[64 * hh + 32:64 * (hh + 1), :],
                                        io[64 * x + 32 * hh:
                                           64 * x + 32 * (hh + 1), :])
                            for th in steps[:2]:
                                th()
                            del steps[:2]

                        # v projection into [t, f] with ones cols interleaved
                        for tl in range(4):
                            tt = 4 * s0 + tl
                            ps_v = ph1ps.tile([128, 256], F32, tag="proj")
                            for kt in range(KT):
                                nc.tensor.matmul(
                                    ps_v, xtq[:, kt, 128 * tl:128 * (tl + 1)],
                                    wv_sb[:, 256 * kt:256 * (kt + 1)],
                                    start=(kt == 0), stop=(kt == KT - 1))
                            # strided evict: head h -> cols [65h, 65h+64)
                            vv = vaug_sb[:, tt, :].rearrange(
                                "p (h d) -> p h d", h=HL)
                            nc.scalar.copy(
                                vv[:, :, 0:64],
                                ps_v.rearrange("p (h d) -> p h d", h=HL))
                        for th in steps:
                            th()

                    for c in range(SC):
                        do_chunk(c)

                # ------------- attention for chunks 1..3 ----------------
                with tc.tile_pool(name="qkps", bufs=1,
                                  space="PSUM") as qkps:
                    def qk_alloc(g):
                        return qkps.tile([128, 1024], F32, tag=f"qk{g}",
                                         bufs=1, name=f"qk_{g}")
                    for i in range(SC * TT):
                        s0, tt = divmod(i, TT)
                        emit_step(s0, tt, qk_alloc)
                    emit_pv(st2["pv"])
                    emit_fin_a(st2["pv"][1])
                    emit_ph3(st2["pv"][1])

    nc.compile()
    return nc


def make_inputs(x, freqs, Wq, Wk, Wv, Wo):
    """Build the 8 per-core input maps."""
    rnd = lambda a: np.ascontiguousarray(a, dtype=ml_dtypes.bfloat16)  # noqa: E731

    # deinterleave permutations of the 256 local feature rows
    p = np.arange(128)
    real_rows = 64 * (p // 32) + 2 * (p % 32)       # within local 256 block
    imag_rows = real_rows + 1

    frh = np.ascontiguousarray(np.tile(freqs[:, :, 0].T, (HL, 1)),
                               dtype=np.float32)    # [128, S]
    fih = np.ascontiguousarray(np.tile(freqs[:, :, 1].T, (HL, 1)),
                               dtype=np.float32)

    def proj_weight(W, rows):
        # lhsT tiles: [128 c-part, KT*128], w[p, kt*128+m] = W[base+rows[m], kt*128+p]
        wt = W[rows, :]                              # [128, C]
        return rnd(wt.T.reshape(KT, 128, 128).transpose(1, 0, 2)
                   .reshape(128, KT * 128))

    in_maps = []
    for c in range(N_CORES):
        b, hg = divmod(c, 4)
        base = 256 * hg
        wqr = proj_weight(Wq, base + real_rows)
        wqi = proj_weight(Wq, base + imag_rows)
        wkr = proj_weight(Wk, base + real_rows)
        wki = proj_weight(Wk, base + imag_rows)
        # v: [128 c-part, KT*256], wv[p, kt*256+f] = Wv[base+f, kt*128+p]
        wvt = Wv[base:base + F, :].T                 # [C, F]
        wv_ = rnd(wvt.reshape(KT, 128, F).transpose(1, 0, 2)
                  .reshape(128, KT * F))
        # wo: [2, 128, C]; pair tile x rows = Wo columns for heads 2x,2x+1
        wo_ = np.empty((2, 128, C), np.float32)
        for xx in range(2):
            wo_[xx] = Wo[:, base + 128 * xx: base + 128 * (xx + 1)].T
        in_maps.append({
            "xt": rnd(x[b].T),
            "ident": rnd(np.eye(128, dtype=np.float32)),
            "wqr": wqr, "wqi": wqi, "wkr": wkr, "wki": wki,
            "wv": wv_, "wo": rnd(wo_),
            "fr": frh, "fi": fih,
        })
    return in_maps


def kernel(x, freqs, Wq, Wk, Wv, Wo):
    global _CACHED_NC
    x = np.asarray(x, dtype=np.float32)
    freqs = np.asarray(freqs, dtype=np.float32)
    Wq = np.asarray(Wq, dtype=np.float32)
    Wk = np.asarray(Wk, dtype=np.float32)
    Wv = np.asarray(Wv, dtype=np.float32)
    Wo = np.asarray(Wo, dtype=np.float32)

    in_maps = make_inputs(x, freqs, Wq, Wk, Wv, Wo)
    if _CACHED_NC is None:
        _CACHED_NC = build_module()
    res = run_bass_kernel_spmd(_CACHED_NC, in_maps,
                               core_ids=list(range(N_CORES)))
    outs = [np.asarray(r["out"], dtype=np.float32) for r in res.results]
    y = np.empty((B, S, C), np.float32)
    for b in range(B):
        y[b] = outs[4 * b] + outs[4 * b + 1] + outs[4 * b + 2] + outs[4 * b + 3]
    return y


if __name__ == "__main__":
    rng = np.random.default_rng(0)
    x = rng.standard_normal((B, S, C)).astype(np.float32)
    freqs = rng.standard_normal((S, DH, 2)).astype(np.float32)
    ws = [(rng.standard_normal((C, C)) * C ** -0.5).astype(np.float32)
          for _ in range(4)]
    y = kernel(x, freqs, *ws)
    print("out", y.shape, y.dtype, float(np.abs(y).mean()))
